# revision 12
# baseline (speedup 1.0000x reference)
"""Trainium2 Bass kernel for nn_Detector (region-sum pooling + softmax).

The reference computes softmax(x.reshape(B, H*W) @ filt) where filt is a
fixed 0/1 mask selecting 10 disjoint 113x113 rectangular regions of the
1024x1024 image.  The dense GEMM is really a sparse pooling: out[b, k]
is the sum of x[b] over region k.  Only ~12% of x is ever needed, so we
DMA exactly the 10 regions per image instead of streaming all 512 MB.

Distribution: data-parallel over batch, 8 NeuronCores x 16 images each.

v3: SDMA descriptors under 512 B pay a 2x read-modify-write penalty
(cost model: latency_multiplier=2 below 512 B; measured 40 ns/desc for
the natural 452 B region-row chunks = exactly 452*2/22.5).  So each
region row is loaded through a 128-column (512 B) window aligned down
to a 64 B DRAM boundary: descriptor rate doubles to 22.8 ns, and the
stream lands on the per-core HBM roofline (~9.25 MB at 358 GB/s).
Multi-queue splitting does NOT help (v2: HWDGE emits 1-desc packets at
~55 GB/s/queue and starves SWDGE; the 16 SDMA engines process packets
serially), so all bulk loads stay on the single SWDGE queue; only the
tiny remainder-row DMAs ride the HWDGE queues in parallel.

Per core and region, one SWDGE (gpsimd) DMA loads rows r0..r0+111 for
all 16 images: DRAM side x[:, r0:r0+112, w0:w0+128]; SBUF side is
[128, 14, 128] with partition = (batch, row-octet).  Both enumerate
elements in the same order (SWDGE needs monotonic APs).  The 113th row
of each region is batched into 5 stepped-slice HWDGE DMAs (regions
sharing a column block lie at uniform row stride).

Compute: VectorE reduces the 113 live columns of each region tile to a
[128, 1] partial; a per-region TensorE matmul with a 0/1 block
indicator [128, 16] contracts the 8 octets per batch into PSUM column
k; VectorE adds the remainder-row partials; ScalarE does the
numerically-stable softmax.
"""

import numpy as np

import concourse.bass as bass
import concourse.tile as tile
from concourse import bacc, mybir
from concourse.bass_utils import run_bass_kernel_spmd

# Problem geometry — fixed by the reference's _build_filter(1024, 1024).
B, H, W = 128, 1024, 1024
S = 113  # min(1024 // 9, 1024 // 7)
REGIONS = [(2, 1), (2, 4), (2, 7), (4, 1), (4, 3), (4, 5), (4, 7), (6, 1), (6, 4), (6, 7)]
K = len(REGIONS)
N_CORES = 8
BPC = B // N_CORES  # images per core
F32 = mybir.dt.float32
OCT, GR = 8, 14  # 112 of the 113 region rows = 8 octets x 14 rows
PADW = 128  # 512 B descriptors (>= 512 avoids the SDMA RMW 2x penalty)


def win(cb):
    """64 B-aligned 128-col window covering col block cb; returns (w0, off)."""
    c0 = cb * S
    w0 = (c0 // 16) * 16
    return w0, c0 - w0


# Remainder-row groups: regions sharing a column block lie at uniform row
# stride, so one stepped-slice DMA covers each group.
#   (row_slice, col_block, [region ks], queue) with row = rb*S + 112.
REM_GROUPS = [
    ((338, 791, 226), 1, [0, 3, 7], "sync"),    # c=1: rows 338,564,790
    ((338, 791, 226), 7, [2, 6, 9], "scalar"),  # c=7
    ((338, 791, 452), 4, [1, 8], "sync"),       # c=4: rows 338,790
    ((564, 565, 1), 3, [4], "scalar"),          # c=3: row 564
    ((564, 565, 1), 5, [5], "sync"),            # c=5
]


def build_nc():
    nc = bacc.Bacc("TRN2", target_bir_lowering=False, debug=False)
    x = nc.declare_dram_parameter("x", [BPC, H, W], F32, isOutput=False)
    blk_d = nc.declare_dram_parameter("blk", [128, BPC], F32, isOutput=False)
    out = nc.declare_dram_parameter("out", [BPC, K], F32, isOutput=True)

    with tile.TileContext(nc) as tc:
        with (
            tc.tile_pool(name="reg", bufs=1) as rpool,
            tc.tile_pool(name="mp", bufs=1) as mpool,
            tc.tile_pool(name="small", bufs=1) as spool,
            tc.tile_pool(name="psum", bufs=1, space=bass.MemorySpace.PSUM) as ppool,
        ):
            # Block indicator: blk[p, b] = 1 iff p // 8 == b (sums octets
            # per batch in the matmul below).  Host-provided — engine
            # memsets can only start at partition 0/32/64/96.
            blk = spool.tile([128, BPC], F32)
            nc.sync.dma_start(out=blk[:], in_=blk_d[:])

            # Bulk loads: rows r0..r0+111 of each region through aligned
            # 512 B windows, all on the SWDGE queue.
            mts = []
            for k, (rb, cb) in enumerate(REGIONS):
                r0 = rb * S
                w0, _ = win(cb)
                mt = rpool.tile([128, GR, PADW], F32, tag=f"mt{k}")
                nc.gpsimd.dma_start(
                    out=mt[:], in_=x[:, r0:r0 + OCT * GR, w0:w0 + PADW]
                )
                mts.append(mt)

            # Remainder rows (r0+112), grouped by column block, on the
            # HWDGE queues (parallel to the SWDGE bulk stream).
            rem_tiles = []  # (tile, j, off, k) per region
            for (ra, rb_, rs), cb, ks, qname in REM_GROUPS:
                w0, off = win(cb)
                rt = spool.tile([BPC, len(ks), PADW], F32, tag=f"rem{cb}")
                eng = getattr(nc, qname)
                eng.dma_start(out=rt[:], in_=x[:, ra:rb_:rs, w0:w0 + PADW])
                for j, k in enumerate(ks):
                    rem_tiles.append((rt, j, off, k))

            # Per-region: VectorE reduce of the 113 live columns ->
            # [128,1]; TensorE matmul with blk contracts octets -> PSUM
            # column k.
            py = ppool.tile([BPC, K], F32)
            for k, (rb, cb) in enumerate(REGIONS):
                _, off = win(cb)
                if k == K - 1:
                    # Last region is the latency tail: split its reduce
                    # across VectorE (rows 0..6) and ScalarE (rows 7..13,
                    # Copy-activation with accum) so both halves run
                    # concurrently; the two matmuls accumulate in PSUM.
                    # 9/5 row split: ScalarE has ~300 ns fixed overhead, so
                    # VectorE takes the larger share for balanced halves.
                    dv = 9
                    mpv = mpool.tile([128, 1], F32, tag="mpv")
                    nc.vector.reduce_sum(
                        out=mpv[:], in_=mts[k][:, 0:dv, off:off + S],
                        axis=mybir.AxisListType.XY,
                    )
                    scr = mpool.tile([128, GR - dv, S], F32, tag="scr")
                    mpa = mpool.tile([128, 1], F32, tag="mpa")
                    nc.scalar.activation(
                        scr[:], mts[k][:, dv:GR, off:off + S],
                        mybir.ActivationFunctionType.Copy, accum_out=mpa[:],
                    )
                    nc.tensor.matmul(py[:, k:k + 1], blk[:], mpv[:], start=True, stop=False)
                    nc.tensor.matmul(py[:, k:k + 1], blk[:], mpa[:], start=False, stop=True)
                else:
                    mp = mpool.tile([128, 1], F32, tag=f"mp{k}")
                    nc.vector.reduce_sum(
                        out=mp[:], in_=mts[k][:, :, off:off + S],
                        axis=mybir.AxisListType.XY,
                    )
                    nc.tensor.matmul(py[:, k:k + 1], blk[:], mp[:], start=True, stop=True)

            # Remainder-row partials -> rpart[:, k].
            rpart = spool.tile([BPC, K], F32)
            for rt, j, off, k in sorted(rem_tiles, key=lambda t: t[3]):
                nc.vector.reduce_sum(
                    out=rpart[:, k:k + 1], in_=rt[:, j, off:off + S],
                    axis=mybir.AxisListType.X,
                )

            ys = spool.tile([BPC, K], F32)
            nc.vector.tensor_add(ys[:], py[:], rpart[:])

            # Softmax over the 10 detectors, batches on partitions.
            m = spool.tile([BPC, 1], F32)
            nc.vector.reduce_max(m[:], ys[:], axis=mybir.AxisListType.X)
            negm = spool.tile([BPC, 1], F32)
            nc.vector.tensor_scalar_mul(negm[:], m[:], -1.0)
            e = spool.tile([BPC, K], F32)
            ssum = spool.tile([BPC, 1], F32)
            # Sum the exponentials on VectorE right after Exp lands: the
            # ScalarE accum_out path issues a second ~280 ns ACT write and
            # delays the DVE reciprocal by an extra engine handoff.
            nc.scalar.activation(
                e[:], ys[:], mybir.ActivationFunctionType.Exp, bias=negm[:],
            )
            nc.vector.reduce_sum(ssum[:], e[:], axis=mybir.AxisListType.X)
            rcp = spool.tile([BPC, 1], F32)
            nc.vector.reciprocal(rcp[:], ssum[:])
            o = spool.tile([BPC, K], F32)
            # Per-partition broadcast multiply on DVE (one less engine hop
            # than ScalarE mul before the output DMA).
            nc.vector.tensor_scalar_mul(o[:], e[:], rcp[:])
            nc.sync.dma_start(out=out[:], in_=o[:], single_packet=True)

    nc.compile()
    return nc


_NC = None


def get_nc():
    global _NC
    if _NC is None:
        _NC = build_nc()
    return _NC


def kernel(x, filt=None, **_unused):
    nc = get_nc()
    x = np.ascontiguousarray(np.asarray(x, dtype=np.float32))
    assert x.shape == (B, H, W), x.shape
    blk = np.repeat(np.eye(BPC, dtype=np.float32), OCT, axis=0)
    in_maps = [
        {"x": x[i * BPC:(i + 1) * BPC], "blk": blk} for i in range(N_CORES)
    ]
    res = run_bass_kernel_spmd(nc, in_maps, list(range(N_CORES)))
    return np.concatenate([r["out"] for r in res.results], axis=0)
